# revision 12
# baseline (speedup 1.0000x reference)
"""ARSLM Trainium2 kernel (nn_ARSLM_56942676410825).

Model: 2-layer gated RNN scan (B=4, T=512, H=512, E=256) -> additive
attention over prefix (cumsum softmax) -> vocab projection (V=50257).

Strategy (8 NeuronCores, no collectives):
  - The recurrent scan is sequential and weight-bandwidth-bound; it is
    replicated on every core (state transposed [dim->partitions, batch->free],
    weights stationary in the PE array, bf16 operands / fp32 accumulate).
  - The vocab projection (logits dominate memory: 412 MB) is sharded
    tensor-parallel over vocab: core c computes logits[:, :, c*6283:(c+1)*6283].
  - Host gathers the vocab slices and returns the full output.

Self-contained: shapes/sharding hardcoded; inputs are the full unsharded
tensors from setup_inputs().
"""
import numpy as np
import ml_dtypes
from contextlib import ExitStack

import concourse.bass as bass
import concourse.mybir as mybir
import concourse.tile as tile
from concourse.bass import ds
from concourse.bass_utils import run_bass_kernel_spmd
from concourse.masks import make_identity

P = 128
B = 4
E = 256
H = 512
V = 50257
T_FULL = 512
N_CORES = 8
V_PAD = 50264          # 8 * 6283
V_SLICE = V_PAD // N_CORES  # 6283
EPS = 1e-5

f32 = mybir.dt.float32
bf16 = mybir.dt.bfloat16
i32 = mybir.dt.int32
AF = mybir.ActivationFunctionType
OP = mybir.AluOpType

NK = {0: 10, 1: 12}    # A-phase contraction k-tiles per layer (1280 / 1536)
NM = 12                # A-phase output m-tiles (c1: 8, g1: 4)
NKB = 8                # B-phase (cand) k-tiles (1024)
NKR = {0: 2, 1: 4}     # r-phase k-tiles (256 / 512)
NKH = 4                # H k-tiles (512)


def _split_multi_waits(nc, max_waits=1):
    """This walrus build allows one sync-wait per instruction; move extra
    waits onto preceding EventSemaphore instructions (same engine => same
    ordering semantics)."""
    n = 0
    for f in nc.m.functions:
        for bb in f.blocks:
            il = bb.instructions  # live list
            i = 0
            while i < len(il):
                inst = il[i]
                si = getattr(inst, "sync_info", None)
                ow = list(si.on_wait) if (si is not None and si.on_wait) else []
                if len(ow) > max_waits:
                    keep, extra = ow[-max_waits:], ow[:-max_waits]
                    j = 0
                    while extra:
                        chunk, extra = extra[:max_waits], extra[max_waits:]
                        w = mybir.InstEventSemaphore(
                            name=f"{inst.name}-wsplit{j}", opcode="EventSemaphore",
                            ins=[], outs=[], engine=inst.engine,
                            sync_info=mybir.SyncInfo(on_wait=chunk, on_update=[]))
                        il.insert(i, w)
                        i += 1
                        j += 1
                    inst.sync_info = mybir.SyncInfo(
                        on_wait=keep, on_update=list(si.on_update or []))
                    n += 1
                i += 1
    return n


def build_module(T=T_FULL, split_waits=True):
    assert T % 32 == 0
    T4 = T * 4
    NCH = T4 // P          # gather chunks
    NVC = (V_SLICE + 511) // 512   # vocab chunks (13, last ragged 139)

    nc = bass.Bass()

    # ---------------- inputs ----------------
    def din(name, shape, dt=bf16):
        return nc.declare_dram_parameter(name, list(shape), dt, isOutput=False)

    emb_table_d = din("emb_table", [V, E], f32)
    ids_d = din("ids", [P, NCH], i32)
    wa_d = {l: din(f"wa{l}", [P, NK[l] * NM * P]) for l in (0, 1)}
    wb_d = {l: din(f"wb{l}", [P, NKB * H]) for l in (0, 1)}
    wr_d = {l: din(f"wr{l}", [P, NKR[l] * H]) for l in (0, 1)}
    wg2_d = {l: din(f"wg2_{l}", [P, NKH]) for l in (0, 1)}
    biasA_d = {l: din(f"biasA{l}", [P, NM], f32) for l in (0, 1)}
    bias2_d = {l: din(f"bias2_{l}", [2, H]) for l in (0, 1)}
    biasr_d = {l: din(f"biasr{l}", [2, H]) for l in (0, 1)}
    bg2_d = {l: din(f"bg2_{l}", [B, 1], f32) for l in (0, 1)}
    wa1_d = din("att_wa1", [P, NKH * NKH * P], f32)
    ba1_d = din("att_ba1", [P, NKH], f32)
    wa2_d = din("att_wa2", [P, NKH])
    ba2_d = din("att_ba2", [1, 1], f32)
    wh_d = din("wh", [P, NKH * V_SLICE])
    bhrep_d = din("bhrep", [P, V_SLICE], f32)

    logits_d = nc.declare_dram_parameter("logits", [T4, V_SLICE], f32, isOutput=True)
    gates_d = nc.declare_dram_parameter("gates", [B, T], f32, isOutput=True)

    with tile.TileContext(nc) as tc, ExitStack() as ctx:
        # ------------- persistent pool -------------
        pc = ctx.enter_context(tc.tile_pool(name="const", bufs=1))

        ident = pc.tile([P, P], f32)
        make_identity(nc, ident[:])
        ones2 = pc.tile([2, P], bf16)
        nc.vector.memset(ones2[:], 1.0)
        ones1f = pc.tile([1, P], f32)
        nc.vector.memset(ones1f[:], 1.0)
        ones_row = pc.tile([B, T], f32)
        nc.vector.memset(ones_row[:], 1.0)
        ones_col = pc.tile([P, T], f32)
        nc.vector.memset(ones_col[:], 1.0)
        eps4 = pc.tile([B, 1], f32)
        nc.vector.memset(eps4[:], EPS)

        histT = pc.tile([P, NKH, T, B], f32)
        gates_sb = pc.tile([B, T], f32)

        st = {l: [pc.tile([P, NKH, B], bf16, name=f'st{l}_{x}') for x in range(2)] for l in (0, 1)}
        hrow = {l: [pc.tile([B, H], f32, name=f'hrow{l}_{x}') for x in range(2)] for l in (0, 1)}
        for l in (0, 1):
            for x in range(2):
                nc.vector.memset(st[l][x][:], 0.0)
                nc.vector.memset(hrow[l][x][:], 0.0)

        ids_sb = pc.tile([P, NCH], i32)
        nc.sync.dma_start(ids_sb[:], ids_d[:])
        biasA = {}
        bias2 = {}
        biasr = {}
        bg2 = {}
        wg2 = {}
        for l in (0, 1):
            biasA[l] = pc.tile([P, NM], f32, name=f'biasA{l}')
            nc.sync.dma_start(biasA[l][:], biasA_d[l][:])
            bias2[l] = pc.tile([2, H], bf16, name=f'bias2{l}')
            nc.sync.dma_start(bias2[l][:], bias2_d[l][:])
            biasr[l] = pc.tile([2, H], bf16, name=f'biasr{l}')
            nc.sync.dma_start(biasr[l][:], biasr_d[l][:])
            bg2[l] = pc.tile([B, 1], f32, name=f'bg2{l}')
            nc.sync.dma_start(bg2[l][:], bg2_d[l][:])
            wg2[l] = pc.tile([P, NKH], bf16, name=f'wg2{l}')
            nc.sync.dma_start(wg2[l][:], wg2_d[l][:])
        wa1_att = pc.tile([P, NKH, NKH, P], f32)
        nc.sync.dma_start(wa1_att[:], wa1_d[:].rearrange("p (a b c) -> p a b c", a=NKH, b=NKH))
        ba1_att = pc.tile([P, NKH], f32)
        nc.sync.dma_start(ba1_att[:], ba1_d[:])
        wa2_att = pc.tile([P, NKH], bf16)
        nc.sync.dma_start(wa2_att[:], wa2_d[:])
        ba2_sb = pc.tile([1, 1], f32)
        nc.sync.dma_start(ba2_sb[:], ba2_d[:])

        # ------------- phases 1+2 share the scan-weight pool -------------
        sw_ctx = ExitStack()
        sw = sw_ctx.enter_context(tc.tile_pool(name="scanw", bufs=1))
        embT = sw.tile([P, 2, T, B], bf16)

        # ------------- phase 1: embedding gather + transpose -------------
        with tc.tile_pool(name="gath", bufs=3) as gp, \
             tc.tile_pool(name="gpsum", bufs=4, space="PSUM") as gps:
            for c in range(NCH):
                g = gp.tile([P, E], f32, tag="g")
                nc.gpsimd.indirect_dma_start(
                    out=g[:], out_offset=None, in_=emb_table_d[:],
                    in_offset=bass.IndirectOffsetOnAxis(ap=ids_sb[:, c:c + 1], axis=0))
                for k in range(2):
                    ps = gps.tile([P, P], f32, tag="tp")
                    nc.tensor.transpose(ps[:], g[:, k * P:(k + 1) * P], ident[:])
                    nc.scalar.activation(
                        out=embT[:].rearrange("p a t b -> p (a t b)")[:, k * T4 + c * P: k * T4 + (c + 1) * P],
                        in_=ps[:], func=AF.Copy)

        # ------------- phase 2: the scan -------------
        with tc.tile_pool(name="sc", bufs=3) as sc, \
             tc.tile_pool(name="pa", bufs=3, space="PSUM") as pa, \
             tc.tile_pool(name="pb", bufs=1, space="PSUM") as pb, \
             tc.tile_pool(name="pr", bufs=1, space="PSUM") as pr, \
             tc.tile_pool(name="pg", bufs=1, space="PSUM") as pg, \
             tc.tile_pool(name="pt", bufs=2, space="PSUM") as pt:

            wa = {}
            wb = {}
            wr = {}
            for l in (0, 1):
                wa[l] = sw.tile([P, NK[l], NM, P], bf16, name=f'wa_sb{l}')
                nc.sync.dma_start(wa[l][:], wa_d[l][:].rearrange("p (k m c) -> p k m c", k=NK[l], m=NM))
                wb[l] = sw.tile([P, NKB, H], bf16, name=f'wb_sb{l}')
                nc.sync.dma_start(wb[l][:], wb_d[l][:].rearrange("p (k c) -> p k c", k=NKB))
                wr[l] = sw.tile([P, NKR[l], H], bf16, name=f'wr_sb{l}')
                nc.sync.dma_start(wr[l][:], wr_d[l][:].rearrange("p (k c) -> p k c", k=NKR[l]))

            def substep(t_rv, pp):
                for l in (0, 1):
                    h1T, h2T = st[l][pp], st[l][1 - pp]
                    # rhs k-tiles of ctx = [h1, h2, inp]
                    rhs_tiles = [h1T[:, k, :] for k in range(NKH)]
                    rhs_tiles += [h2T[:, k, :] for k in range(NKH)]
                    if l == 0:
                        xT_s = sc.tile([P, 2, B], bf16, tag="xTs")
                        nc.vector.tensor_copy(out=xT_s[:],
                                              in_=embT[:, :, ds(t_rv, 1), :])
                        inp_tiles = [xT_s[:, k, :] for k in range(2)]
                    else:
                        inp_tiles = [st[0][1 - pp][:, k, :] for k in range(NKH)]
                    rhs_tiles += inp_tiles

                    reluT = sc.tile([P, NKB, B], bf16, tag="reluT")
                    tanhT = sc.tile([P, NKH, B], bf16, tag="tanhT")
                    # --- A phase (weights stationary) ---
                    for m in range(NM):
                        ps = pa.tile([P, B], f32, tag="pa")
                        nk = NK[l]
                        for k in range(nk):
                            nc.tensor.matmul(
                                out=ps[:], lhsT=wa[l][:, k, m, :], rhs=rhs_tiles[k],
                                start=(k == 0), stop=(k == nk - 1))
                        if m < NKB:
                            nc.scalar.activation(out=reluT[:, m, :], in_=ps[:],
                                                 func=AF.Relu, bias=biasA[l][:, m:m + 1])
                        else:
                            nc.scalar.activation(out=tanhT[:, m - NKB, :], in_=ps[:],
                                                 func=AF.Tanh, bias=biasA[l][:, m:m + 1])

                    # --- B phase: cand [4, 512] (state stationary) ---
                    ps_c = pb.tile([B, H], f32, tag="pb")
                    for k in range(NKB):
                        nc.tensor.matmul(out=ps_c[:], lhsT=reluT[:, k, :], rhs=wb[l][:, k, :],
                                         start=(k == 0), stop=False)
                    nc.tensor.matmul(out=ps_c[:], lhsT=ones2[:, :B], rhs=bias2[l][:],
                                     start=False, stop=True)

                    # --- r phase: 0.1*(inp @ Wr + br) [4, 512] ---
                    ps_r = pr.tile([B, H], f32, tag="pr")
                    for k in range(NKR[l]):
                        nc.tensor.matmul(out=ps_r[:], lhsT=inp_tiles[k], rhs=wr[l][:, k, :],
                                         start=(k == 0), stop=False)
                    nc.tensor.matmul(out=ps_r[:], lhsT=ones2[:, :B], rhs=biasr[l][:],
                                     start=False, stop=True)

                    # --- gate: sigmoid(tanh_out @ Wg2 + bg2) [4, 1] ---
                    ps_g = pg.tile([B, 1], f32, tag="pg")
                    for k in range(NKH):
                        nc.tensor.matmul(out=ps_g[:], lhsT=tanhT[:, k, :], rhs=wg2[l][:, k:k + 1],
                                         start=(k == 0), stop=(k == NKH - 1))
                    if l == 1:
                        gate_ap = gates_sb[:, ds(t_rv, 1)]
                    else:
                        g0 = sc.tile([B, 1], f32, tag="g0")
                        gate_ap = g0[:]
                    nc.scalar.activation(out=gate_ap, in_=ps_g[:], func=AF.Sigmoid,
                                         bias=bg2[l][:, 0:1])

                    # --- h_pre = h1 + gate*cand + r ---
                    hr = sc.tile([B, H], f32, tag="hr")
                    nc.vector.tensor_tensor(out=hr[:], in0=hrow[l][pp][:], in1=ps_r[:], op=OP.add)
                    hpre = sc.tile([B, H], f32, tag="hpre")
                    nc.vector.scalar_tensor_tensor(
                        out=hpre[:], in0=ps_c[:], scalar=gate_ap, in1=hr[:],
                        op0=OP.mult, op1=OP.add)

                    # --- LN ---
                    s_m = sc.tile([B, 1], f32, tag="s_m")
                    ssq = sc.tile([B, 1], f32, tag="ssq")
                    sq = sc.tile([B, H], f32, tag="sq")
                    nc.vector.tensor_reduce(out=s_m[:], in_=hpre[:], axis=mybir.AxisListType.X, op=OP.add)
                    nc.vector.tensor_scalar_mul(s_m[:], s_m[:], 1.0 / H)
                    nc.vector.tensor_tensor(out=sq[:], in0=hpre[:], in1=hpre[:], op=OP.mult)
                    nc.vector.tensor_reduce(out=ssq[:], in_=sq[:], axis=mybir.AxisListType.X, op=OP.add)
                    nc.vector.tensor_scalar_mul(ssq[:], ssq[:], 1.0 / H)
                    msq = sc.tile([B, 1], f32, tag="msq")
                    nc.vector.tensor_tensor(out=msq[:], in0=s_m[:], in1=s_m[:], op=OP.mult)
                    var = sc.tile([B, 1], f32, tag="var")
                    nc.vector.scalar_tensor_tensor(out=var[:], in0=ssq[:], scalar=1.0, in1=msq[:],
                                                   op0=OP.mult, op1=OP.subtract)
                    std = sc.tile([B, 1], f32, tag="std")
                    nc.scalar.activation(out=std[:], in_=var[:], func=AF.Sqrt, bias=eps4[:, 0:1])
                    istd = sc.tile([B, 1], f32, tag="istd")
                    nc.vector.reciprocal(out=istd[:], in_=std[:])
                    hnew = hrow[l][1 - pp]
                    nc.vector.tensor_scalar(
                        out=hnew[:], in0=hpre[:], scalar1=s_m[:, 0:1], scalar2=istd[:, 0:1],
                        op0=OP.subtract, op1=OP.mult)

                    # --- transpose h_new -> [128, 4] tiles ---
                    hstage = sc.tile([P, NKH, B], f32, tag="hstage", name="hstage") if l == 1 else None
                    for k in range(NKH):
                        pst = pt.tile([P, B], f32, tag="pt")
                        nc.tensor.transpose(pst[:], hnew[:, k * P:(k + 1) * P], ident[:B, :B])
                        nc.scalar.activation(out=st[l][1 - pp][:, k, :], in_=pst[:], func=AF.Copy)
                        if l == 1:
                            nc.vector.tensor_copy(out=hstage[:, k, :], in_=pst[:])
                    if l == 1:
                        nc.vector.tensor_copy(out=histT[:, :, ds(t_rv, 1), :], in_=hstage[:])

            with tc.For_i(0, T, 2) as i:
                substep(i, 0)
                substep(i + 1, 1)

        sw_ctx.close()   # release scan weights + embT

        # ------------- phase 3: attention -------------
        po_ctx = ExitStack()
        po = po_ctx.enter_context(tc.tile_pool(name="post", bufs=1))
        attT_bf = po.tile([P, NKH, T4], bf16)
        bhrep = po.tile([P, V_SLICE], f32)
        nc.sync.dma_start(bhrep[:], bhrep_d[:])

        with tc.tile_pool(name="att", bufs=1) as at, \
             tc.tile_pool(name="apsum", bufs=4, space="PSUM") as aps, \
             tc.tile_pool(name="apsum1", bufs=2, space="PSUM") as aps1:

            # pre-scores: tanh(hist @ Wa1 + ba1), T-form [128(4m), T4] (fp32 matmul)
            nch_att = (T4 + 511) // 512
            def csl(chs):
                sz = min(512, T4 - chs * 512)
                return slice(chs * 512, chs * 512 + sz), sz
            hist_flat = histT[:].rearrange("p k t b -> p k (t b)")
            tanhT_bf = at.tile([P, NKH, T4], bf16)
            for m in range(NKH):
                for chs in range(nch_att):
                    cs, sz = csl(chs)
                    ps = aps.tile([P, 512], f32, tag="aps")
                    for k in range(NKH):
                        nc.tensor.matmul(out=ps[:, :sz], lhsT=wa1_att[:, k, m, :],
                                         rhs=hist_flat[:, k, cs],
                                         start=(k == 0), stop=(k == NKH - 1))
                    nc.scalar.activation(out=tanhT_bf[:, m, cs],
                                         in_=ps[:, :sz], func=AF.Tanh, bias=ba1_att[:, m:m + 1])

            # scores [1, T4] = tanh_pre @ Wa2 + ba2
            scores = at.tile([1, T4], f32)
            for chs in range(nch_att):
                cs, sz = csl(chs)
                ps1 = aps1.tile([1, 512], f32, tag="aps1")
                for k in range(NKH):
                    nc.tensor.matmul(out=ps1[:, :sz], lhsT=wa2_att[:, k:k + 1],
                                     rhs=tanhT_bf[:, k, cs],
                                     start=(k == 0), stop=(k == NKH - 1))
                nc.vector.tensor_scalar_add(scores[:, cs], ps1[:, :sz], ba2_sb[:, 0:1])

            # e = exp(s - max_t s) ; layout j = t*4+b
            sc_bt = scores[:].rearrange("p (t b) -> p b t", b=B)
            m_b = at.tile([1, B], f32)
            nc.vector.tensor_reduce(out=m_b[:].unsqueeze(2), in_=sc_bt,
                                    axis=mybir.AxisListType.X, op=OP.max)
            e_sb = at.tile([1, T4], f32)
            nc.vector.tensor_tensor(out=e_sb[:].rearrange("p (t b) -> p b t", b=B),
                                    in0=sc_bt,
                                    in1=m_b[:].unsqueeze(2).to_broadcast([1, B, T]),
                                    op=OP.subtract)
            nc.scalar.activation(out=e_sb[:], in_=e_sb[:], func=AF.Exp)

            # den cumsum per b, then reciprocal
            den = at.tile([1, T4], f32)
            e_bt = e_sb[:].rearrange("p (t b) -> p b t", b=B)
            den_bt = den[:].rearrange("p (t b) -> p b t", b=B)
            for b in range(B):
                nc.vector.tensor_tensor_scan(
                    out=den_bt[:, b, :], data0=ones_row[:1, :T], data1=e_bt[:, b, :],
                    initial=0.0, op0=OP.mult, op1=OP.add)
            recip = at.tile([1, T4], f32)
            nc.vector.reciprocal(out=recip[:], in_=den[:])

            # broadcast e and recip across partitions (fp32 ones matmul)
            ebc = at.tile([P, T4], f32)
            rbc = at.tile([P, T4], f32)
            for chs in range(nch_att):
                cs, sz = csl(chs)
                ps = aps.tile([P, 512], f32, tag="aps")
                nc.tensor.matmul(out=ps[:, :sz], lhsT=ones1f[:], rhs=e_sb[:, cs], start=True, stop=True)
                nc.scalar.activation(out=ebc[:, cs], in_=ps[:, :sz], func=AF.Copy)
                ps2 = aps.tile([P, 512], f32, tag="aps")
                nc.tensor.matmul(out=ps2[:, :sz], lhsT=ones1f[:], rhs=recip[:, cs], start=True, stop=True)
                nc.scalar.activation(out=rbc[:, cs], in_=ps2[:, :sz], func=AF.Copy)

            # num = cumsum_t(e*hist); attended = hist + num*recip
            numT = at.tile([P, NKH, T4], f32)
            for k in range(NKH):
                nc.vector.tensor_tensor(out=numT[:, k, :], in0=histT[:, k, :, :].rearrange("p t b -> p (t b)"),
                                        in1=ebc[:], op=OP.mult)
            for k in range(NKH):
                nt_bt = numT[:, k, :].rearrange("p (t b) -> p b t", b=B)
                for b in range(B):
                    nc.vector.tensor_tensor_scan(
                        out=nt_bt[:, b, :], data0=ones_col[:, :T],
                        data1=nt_bt[:, b, :], initial=0.0, op0=OP.mult, op1=OP.add)
            for k in range(NKH):
                nc.vector.tensor_tensor(out=numT[:, k, :], in0=numT[:, k, :], in1=rbc[:], op=OP.mult)
                nc.vector.tensor_tensor(out=numT[:, k, :], in0=numT[:, k, :],
                                        in1=histT[:, k, :, :].rearrange("p t b -> p (t b)"), op=OP.add)
                nc.scalar.activation(out=attT_bf[:, k, :], in_=numT[:, k, :], func=AF.Copy)

        # ------------- phase 4: vocab head (wh streamed per vocab chunk) -------------
        wh_v = wh_d[:].rearrange("p (k v) -> p k v", k=NKH)
        with tc.tile_pool(name="hwht", bufs=2) as hw, \
             tc.tile_pool(name="hout", bufs=4) as ho, \
             tc.tile_pool(name="hpsum", bufs=4, space="PSUM") as hp:
            for n in range(NVC):
                nsz = min(512, V_SLICE - n * 512)
                vs = slice(n * 512, n * 512 + nsz)
                wht = hw.tile([P, NKH, 512], bf16, tag="wht")
                nc.sync.dma_start(wht[:, :, :nsz], wh_v[:, :, vs])
                for c in range(T4 // P):
                    ps = hp.tile([P, 512], f32, tag="hp")
                    for k in range(NKH):
                        nc.tensor.matmul(out=ps[:, :nsz], lhsT=attT_bf[:, k, c * P:(c + 1) * P],
                                         rhs=wht[:, k, :nsz], start=(k == 0), stop=(k == NKH - 1))
                    ot = ho.tile([P, 512], f32, tag="ot")
                    nc.vector.tensor_tensor(out=ot[:, :nsz], in0=ps[:, :nsz],
                                            in1=bhrep[:, vs], op=OP.add)
                    nc.sync.dma_start(logits_d[c * P:(c + 1) * P, vs], ot[:, :nsz])
        po_ctx.close()

        nc.sync.dma_start(gates_d[:], gates_sb[:])

    nc.finalize()
    if split_waits:
        _split_multi_waits(nc)
    return nc


# ---------------- host-side prep ----------------

def _bf(x):
    return np.asarray(x, dtype=np.float32).astype(ml_dtypes.bfloat16)


def _hi_lo(b):
    hi = _bf(b)
    lo = _bf(np.asarray(b, np.float32) - hi.astype(np.float32))
    return np.stack([hi, lo], axis=0)  # [2, N]


def _stationary(w, nk, nm, dtype=None):
    # [K, M] -> [128, nk, nm, 128] -> [128, nk*nm*128]
    w = _bf(w) if dtype is None else np.asarray(w, np.float32)
    return np.ascontiguousarray(
        w.reshape(nk, P, nm, P).transpose(1, 0, 2, 3).reshape(P, nk * nm * P))


def _rhs_form(w, nk):
    # [K, N] -> [128, nk*N]
    w = _bf(w)
    n = w.shape[1]
    return np.ascontiguousarray(w.reshape(nk, P, n).transpose(1, 0, 2).reshape(P, nk * n))


def _col_form(w):
    # [K, 1] -> [128, K/128]
    w = _bf(np.asarray(w, np.float32).reshape(-1))
    return np.ascontiguousarray(w.reshape(-1, P).T)


def prepare_inputs(inputs, T=T_FULL):
    inp = {k: np.asarray(v) for k, v in inputs.items()}
    ids = inp["input_ids"].astype(np.int32)[:, :T]          # [4, T]
    NCH = T * 4 // P
    ids_tm = np.ascontiguousarray(ids.T.reshape(-1))        # j = t*4+b
    ids_sb = np.ascontiguousarray(ids_tm.reshape(NCH, P).T)

    common = {
        "emb_table": np.ascontiguousarray(inp["emb_table"], dtype=np.float32),
        "ids": ids_sb,
    }
    for l in (0, 1):
        wc1, bc1 = inp[f"Wc1_{l}"], inp[f"bc1_{l}"]
        wg1, bg1 = inp[f"Wg1_{l}"], inp[f"bg1_{l}"]
        wc2, bc2 = inp[f"Wc2_{l}"], inp[f"bc2_{l}"]
        wg2, bg2 = inp[f"Wg2_{l}"], inp[f"bg2_{l}"]
        wr, br = inp[f"Wr_{l}"], inp[f"br_{l}"]
        wa = np.concatenate([wc1, wg1], axis=1)             # [K, 1536]
        common[f"wa{l}"] = _stationary(wa, NK[l], NM)
        common[f"wb{l}"] = _rhs_form(wc2, NKB)
        common[f"wr{l}"] = _rhs_form(0.1 * wr, NKR[l])
        common[f"wg2_{l}"] = _col_form(wg2)
        common[f"biasA{l}"] = np.ascontiguousarray(
            np.concatenate([bc1, bg1]).astype(np.float32).reshape(NM, P).T)
        common[f"bias2_{l}"] = _hi_lo(bc2)
        common[f"biasr{l}"] = _hi_lo(0.1 * np.asarray(br, np.float32))
        common[f"bg2_{l}"] = np.full((B, 1), np.asarray(bg2, np.float32).reshape(-1)[0], np.float32)

    common["att_wa1"] = _stationary(inp["Wa1"], NKH, NKH, dtype="f32")
    common["att_ba1"] = np.ascontiguousarray(inp["ba1"].astype(np.float32).reshape(NKH, P).T)
    common["att_wa2"] = _col_form(inp["Wa2"])
    common["att_ba2"] = np.asarray(inp["ba2"], np.float32).reshape(1, 1)

    wh_pad = np.zeros((H, V_PAD), np.float32)
    wh_pad[:, :V] = np.asarray(inp["Wh"], np.float32)
    bh_pad = np.zeros((V_PAD,), np.float32)
    bh_pad[:V] = np.asarray(inp["bh"], np.float32)

    in_maps = []
    for c in range(N_CORES):
        vs = slice(c * V_SLICE, (c + 1) * V_SLICE)
        m = dict(common)
        m["wh"] = _rhs_form(wh_pad[:, vs], NKH)
        m["bhrep"] = np.ascontiguousarray(
            np.broadcast_to(bh_pad[vs][None, :], (P, V_SLICE)).astype(np.float32))
        in_maps.append(m)
    return in_maps


def assemble_outputs(results, T=T_FULL):
    slices = []
    for c in range(N_CORES):
        lg = results[c]["logits"]                      # [T*4, V_SLICE], rows j=t*4+b
        slices.append(lg.reshape(T, B, V_SLICE).transpose(1, 0, 2))
    logits = np.concatenate(slices, axis=2)[:, :, :V]  # [4, T, V]
    gates = np.ascontiguousarray(results[0]["gates"])  # [4, T]
    return np.ascontiguousarray(logits), gates


_module_cache = {}


def kernel(**inputs):
    T = T_FULL
    if T not in _module_cache:
        _module_cache[T] = build_module(T)
    nc = _module_cache[T]
    in_maps = prepare_inputs(inputs, T)
    res = run_bass_kernel_spmd(nc, in_maps, core_ids=list(range(N_CORES)))
    return assemble_outputs(res.results, T)
